# revision 29
# baseline (speedup 1.0000x reference)
"""Trainium2 Bass kernel for nn_Attention_5609227288590 (sparse_attention).

Math: the reference's suppress branch with THRES=1.0 has an all-True mask
(every attn value <= its row max), so it reduces exactly to

    attn' = suppress * attn^2 / (rowsum(attn) + 1e-6)

with rowsum(attn) == 1 up to fp rounding of the softmax itself.  Writing
P = exp(S) (no max subtraction needed: |S| <= ~4 for this distribution),
Z_i = sum_j P_ij:

    out_head[i, :] = c * (P∘P) @ V / Z_i^2 ,   c = suppress / (1 + 1e-6)

Per-core layout (data-parallel over batch, 2 batches/core):
  - qT (channels x tokens) computed with w_qkv as stationary, x^T as moving.
    The softmax scale is folded into the Q weights host-side.
  - K^T is stored ZERO-PADDED to K=128: per head pair, two [128, T] tiles
    (rows 0:64 = even head's K^T over zeros / rows 64:128 = odd head's over
    zeros).  S^T matmuls then run with a full-K=128 stationary: measured
    216ns/512-col vs 427+ for the K=64 form (the PE streams K<=64
    stationaries at half rate).
  - V computed in (tokens x channels) layout -> V slices are direct lhsT for
    the PV matmul; P2^T is the moving operand (contraction j on partitions).
  - Z by ones[128,64]-stationary matmuls col-packed 2 heads/bank, giving Z
    broadcast across 64 partitions, matching the PV psum layout.  (gpsimd
    partition_all_reduce measured 13us/[128,2048] on HW - unusable.)
  - Exp(S) is the ONLY ScalarE work; P^2 is one fused [128,2048] DVE square
    per jt, and 1/Z^2 is DVE reciprocal_approx_fast + two multiplies.
  - Slots run BATCH-OUTER so each batch's attn-out completes mid-kernel;
    its out-projection token-tiles are injected one-per-slot into the
    pipeline (psum from the "s" tag ring, keeping ring parity) so only the
    last batch's 8 tiles remain as tail.
  - ~12 warmup matmuls on a zeroed tile run during the input-DMA window so
    the PE p-state is at full clock (2.4GHz) when real work arrives.

Measured (full clock): PE busy ~373us vs a 369us bf16 column-streaming
floor; exec ~414-416us.  Dead ends verified on HW: gpsimd
partition_all_reduce 13us/[128,2048]; any concurrent gpsimd bulk op slows
DVE ~7x (SBUF contention); bf16->fp8 casts for DoubleRow-Z exceed DVE
slack; fp8 anywhere upstream of exp or in P^2/V costs ~3.5% output error
vs the 2e-2 budget.
"""

import numpy as np
import ml_dtypes

import concourse.bass as bass
import concourse.mybir as mybir
import concourse.tile as tile
from concourse import bacc
from concourse.bass_utils import run_bass_kernel_spmd

BF16 = mybir.dt.bfloat16
F32 = mybir.dt.float32
AF = mybir.ActivationFunctionType

N_CORES = 8
B = 16
N = 1024
DIM = 768
HEADS = 12
DH = 64
B_PC = B // N_CORES          # 2 batches per core
T = B_PC * N                 # 2048 tokens per core
PAIRS = HEADS // 2           # 6 head pairs
KT = DIM // 128              # 6 contraction tiles for projections
SCALE = DH ** -0.5           # 0.125
JTS = N // 128               # 8 j-tiles per attention step

LAST_RESULTS = None  # BassKernelResults of the last run (for test.py)


def _patch_act_tables():
    """Pin all activations to the natural_log_exp_and_others table set."""
    import concourse.hw_specs as hw_specs

    if getattr(bacc, "_act_tables_patched", False):
        return
    orig = hw_specs.get_activation_tables

    def patched(module_arch):
        tabs = orig(module_arch)
        return {
            name: (funcs if name == "natural_log_exp_and_others" else set())
            for name, funcs in tabs.items()
        }

    bacc.get_activation_tables = patched
    bacc._act_tables_patched = True


def _build_kernel():
    _patch_act_tables()
    nc = bacc.Bacc("TRN2", target_bir_lowering=False, debug=False)

    xT = nc.dram_tensor("xT", [DIM, T], BF16, kind="ExternalInput")
    w_qk = nc.dram_tensor("w_qk", [DIM, 2 * DIM], BF16, kind="ExternalInput")
    w_v = nc.dram_tensor("w_v", [DIM, DIM], BF16, kind="ExternalInput")
    w_out = nc.dram_tensor("w_out", [DIM, DIM], BF16, kind="ExternalInput")
    b_out = nc.dram_tensor("b_out", [1, DIM], BF16, kind="ExternalInput")
    out = nc.dram_tensor("out", [T, DIM], F32, kind="ExternalOutput")
    import os
    dbg = None
    if os.environ.get("KDEBUG") == "1":
        dbg = {
            "dbg_q": nc.dram_tensor("dbg_q", [128, PAIRS * T], BF16, kind="ExternalOutput"),
            "dbg_k": nc.dram_tensor("dbg_k", [128, 2 * PAIRS * T], BF16, kind="ExternalOutput"),
            "dbg_v": nc.dram_tensor("dbg_v", [128, (T // 128) * DIM], BF16, kind="ExternalOutput"),
            "dbg_ao": nc.dram_tensor("dbg_ao", [128, KT * T], BF16, kind="ExternalOutput"),
        }

    with tile.TileContext(nc) as tc:
        _body(nc, tc, xT, w_qk, w_v, w_out, b_out, out, dbg)
    nc.compile()
    return nc


def _body(nc, tc, xT, w_qk, w_v, w_out, b_out, out, dbg=None):
    from contextlib import ExitStack

    ctx = ExitStack()
    with ctx:
        singles = ctx.enter_context(tc.tile_pool(name="singles", bufs=1))

        # ---- persistent SBUF tensors ----
        w_v_sb = singles.tile([128, KT, DIM], BF16)
        w_out_sb = singles.tile([128, KT, DIM], BF16)
        b_out_sb = singles.tile([1, DIM], BF16)
        bias_bcast = singles.tile([128, DIM], BF16)
        ones64 = singles.tile([128, DH], BF16)
        ones1 = singles.tile([1, 128], BF16)
        qT_sb = singles.tile([128, PAIRS, T], BF16)
        # zero-padded K^T: per pair, index 2h   = [K_h0^T (rows 0:64); 0]
        #                            index 2h+1 = [0; K_h1^T (rows 64:128)]
        kpad_sb = singles.tile([128, 2 * PAIRS, T], BF16)
        v_sb = singles.tile([128, T // 128, DIM], BF16)    # [t, c] layout
        aoT_sb = singles.tile([128, KT, T], BF16)          # attn-outT stacked

        w_qk_r = w_qk.rearrange("(ko p) c -> p ko c", p=128)
        w_v_r = w_v.rearrange("(ko p) c -> p ko c", p=128)
        xt_r = xT.rearrange("(ko p) t -> p ko t", p=128)

        # ---- phase 1+2: projections ----
        with (
            tc.tile_pool(name="proj_in", bufs=1) as proj_in,
            tc.tile_pool(name="proj_ps", bufs=2, space="PSUM") as proj_ps,
        ):
            w_qk_sb = proj_in.tile([128, KT, 2 * DIM], BF16)
            xt_sb = proj_in.tile([128, KT, T], BF16)
            warm = proj_in.tile([128, 640], BF16)
            # zero-fills + warmup first: no input deps, run during DMA wait
            nc.any.memset(warm, 0.0)
            nc.any.memset(ones64, 1.0)
            nc.any.memset(ones1, 1.0)
            for h in range(PAIRS):
                nc.gpsimd.memset(kpad_sb[64:128, 2 * h, :], 0.0)
                nc.gpsimd.memset(kpad_sb[0:64, 2 * h + 1, :], 0.0)
            ps_w = proj_ps.tile([128, T], F32, tag="proj", name="ps_w")
            for r in range(12):
                nc.tensor.matmul(
                    ps_w[:, (r % 4) * 512:(r % 4) * 512 + 512],
                    warm[:, 0:128],
                    warm[:, 128:640],
                    start=True,
                    stop=True,
                )
            # DMA order: first QK weight block, then x chunks (kt-major so
            # the kt-inner projection loop starts as chunks land), then rest.
            nc.sync.dma_start(w_qk_sb[:, :, 0:128], w_qk_r[:, :, 0:128])
            nc.sync.dma_start(xt_sb[:, 0, 0:1024], xt_r[:, 0, 0:1024])
            nc.sync.dma_start(w_qk_sb[:, :, 128:640], w_qk_r[:, :, 128:640])
            nc.sync.dma_start(xt_sb[:, 0, 1024:2048], xt_r[:, 0, 1024:2048])
            for kt in range(1, KT):
                nc.sync.dma_start(xt_sb[:, kt, :], xt_r[:, kt, :])
            nc.sync.dma_start(w_qk_sb[:, :, 640:], w_qk_r[:, :, 640:])
            nc.sync.dma_start(w_v_sb, w_v_r)
            nc.sync.dma_start(
                w_out_sb, w_out.rearrange("(ko p) c -> p ko c", p=128)
            )
            nc.sync.dma_start(b_out_sb, b_out[:, :])

            for mt in range(2 * PAIRS):
                ps = proj_ps.tile([128, T], F32, tag="proj")
                for kt in range(KT):
                    for c in range(T // 512):
                        nc.tensor.matmul(
                            ps[:, c * 512:(c + 1) * 512],
                            w_qk_sb[:, kt, mt * 128:(mt + 1) * 128],
                            xt_sb[:, kt, c * 512:(c + 1) * 512],
                            start=(kt == 0),
                            stop=(kt == KT - 1),
                        )
                if mt < PAIRS:
                    nc.vector.tensor_copy(out=qT_sb[:, mt, :], in_=ps)
                else:
                    h = mt - PAIRS
                    nc.vector.tensor_copy(
                        out=kpad_sb[0:64, 2 * h, :], in_=ps[0:64, :]
                    )
                    nc.vector.tensor_copy(
                        out=kpad_sb[64:128, 2 * h + 1, :], in_=ps[64:128, :]
                    )

            for mt in range(T // 128):
                ps = proj_ps.tile([128, DIM], F32, tag="proj")
                for kt in range(KT):
                    for c0, c1 in ((0, 512), (512, 768)):
                        nc.tensor.matmul(
                            ps[:, c0:c1],
                            xt_sb[:, kt, mt * 128:(mt + 1) * 128],
                            w_v_sb[:, kt, c0:c1],
                            start=(kt == 0),
                            stop=(kt == KT - 1),
                        )
                nc.vector.tensor_copy(out=v_sb[:, mt, :], in_=ps)

            # broadcast the output bias across partitions once (K=1 matmul)
            ps_b = proj_ps.tile([128, DIM], F32, tag="proj")
            for c0, c1 in ((0, 512), (512, 768)):
                nc.tensor.matmul(
                    ps_b[:, c0:c1],
                    ones1[0:1, 0:128],
                    b_out_sb[0:1, c0:c1],
                    start=True,
                    stop=True,
                )
            nc.vector.tensor_copy(out=bias_bcast, in_=ps_b)

        # ---- phase 3: attention, per (batch, head pair) + fused out-proj ----
        with (
            tc.tile_pool(name="pt_pool", bufs=8) as pt_pool,
            tc.tile_pool(name="p2t_pool", bufs=5) as p2t_pool,
            tc.tile_pool(name="z_sb_pool", bufs=1) as z_sb_pool,
            tc.tile_pool(name="f_sb", bufs=2) as f_sb,
            tc.tile_pool(name="s_ps", bufs=2, space="PSUM") as s_ps,
            tc.tile_pool(name="o_ps", bufs=1, space="PSUM") as o_ps,
            tc.tile_pool(name="z_ps", bufs=1, space="PSUM") as z_ps,
        ):
            pts, p2ts = {}, {}
            obank = {}  # (h, b) -> (psum_o, psum_z)

            def s_exp_sq(h, b, jt):
                t0 = b * N
                qT = qT_sb[:, h, t0:t0 + N]
                pt = pt_pool.tile([128, 2 * N], BF16, tag="pt")
                p2t = p2t_pool.tile([128, 2 * N], BF16, tag="p2t")
                pts[(h, b, jt)], p2ts[(h, b, jt)] = pt, p2t
                with tc.high_priority(offset=140):
                    for hh in (1, 0):
                        kp = kpad_sb[
                            :, 2 * h + hh, t0 + jt * 128:t0 + (jt + 1) * 128
                        ]
                        ps = s_ps.tile([128, N], F32, tag="s", name="psS")
                        for c in range(2):
                            nc.tensor.matmul(
                                ps[:, c * 512:(c + 1) * 512],
                                kp,
                                qT[:, c * 512:(c + 1) * 512],
                                start=True,
                                stop=True,
                            )
                        nc.scalar.activation(
                            pt[:, hh * N:(hh + 1) * N], ps, AF.Exp,
                        )
                    # one fused square for both head halves (DVE 2x bf16)
                    nc.vector.tensor_mul(out=p2t, in0=pt, in1=pt)

            def zpv(h, b, jt):
                # skip_group_check: the sim's global zero-region check
                # mishandles base_partition!=0; col-split groups are
                # HW-safe (verified by direct probe).
                if jt == 0:
                    obank[(h, b)] = (
                        o_ps.tile([128, 1024], F32, tag="o", name="psum_o"),
                        z_ps.tile([128, 1024], F32, tag="z", name="psum_z"),
                    )
                psum_o, psum_z = obank[(h, b)]
                pt, p2t = pts.pop((h, b, jt)), p2ts.pop((h, b, jt))
                vt = v_sb[:, b * 8 + jt, :]
                first, last = jt == 0, jt == JTS - 1
                for hh in (1, 0):
                    d0, d1 = hh * 64, hh * 64 + 64
                    ch0 = h * 128 + hh * 64
                    for c in range(2):
                        nc.tensor.matmul(
                            psum_z[d0:d1, c * 512:(c + 1) * 512],
                            ones64,
                            pt[:, hh * N + c * 512:hh * N + (c + 1) * 512],
                            start=first,
                            stop=last,
                            skip_group_check=True,
                        )
                    for c in range(2):
                        nc.tensor.matmul(
                            psum_o[d0:d1, c * 512:(c + 1) * 512],
                            vt[:, ch0:ch0 + 64],
                            p2t[:, hh * N + c * 512:hh * N + (c + 1) * 512],
                            start=first,
                            stop=last,
                            skip_group_check=True,
                        )
                if last:
                    psum_o, psum_z = obank.pop((h, b))
                    zinv = z_sb_pool.tile([128, 1024], F32, tag="zinv")
                    otmp = z_sb_pool.tile([128, 1024], BF16, tag="otmp")
                    with tc.high_priority(offset=90):
                        nc.vector.reciprocal_approx_fast(zinv, psum_z)
                        nc.vector.tensor_mul(out=otmp, in0=psum_o, in1=zinv)
                        nc.vector.tensor_mul(
                            out=aoT_sb[:, h, b * N:(b + 1) * N],
                            in0=otmp,
                            in1=zinv,
                        )

            def out_proj(mt):
                # one token-tile of the final projection; psum comes from the
                # "s" tag ring so psum stays within 8 banks (parity: exactly
                # one of these per pipeline step).
                ps = s_ps.tile([128, N], F32, tag="s", name="psF")
                for c0, c1 in ((0, 512), (512, 768)):
                    for kt in range(KT):
                        nc.tensor.matmul(
                            ps[:, c0:c1],
                            aoT_sb[:, kt, mt * 128:(mt + 1) * 128],
                            w_out_sb[:, kt, c0:c1],
                            start=(kt == 0),
                            stop=(kt == KT - 1),
                        )
                o_sb = f_sb.tile([128, DIM], F32, tag="fo")
                nc.vector.tensor_add(out=o_sb, in0=ps[:, 0:DIM], in1=bias_bcast)
                nc.sync.dma_start(out[mt * 128:(mt + 1) * 128, :], o_sb)

            # global slot pipeline, batch-outer: zpv lags s_exp_sq by LAG
            # slots; once a batch's last zpv ran, its out-proj token tiles
            # are fed one-per-step into the same pipeline.
            LAG = 2
            slots = [
                (h, b, jt)
                for b in range(B_PC)
                for h in range(PAIRS)
                for jt in range(JTS)
            ]
            pending_f = []  # out-proj token tiles ready to issue
            for k in range(len(slots) + LAG):
                if k >= LAG:
                    h, b, jt = slots[k - LAG]
                    zpv(h, b, jt)
                    if h == PAIRS - 1 and jt == JTS - 1:
                        pending_f.extend(range(b * 8, (b + 1) * 8))
                if k < len(slots):
                    s_exp_sq(*slots[k])
                if pending_f:
                    out_proj(pending_f.pop(0))
            while pending_f:
                out_proj(pending_f.pop(0))
            if dbg is not None:
                nc.sync.dma_start(dbg["dbg_q"][:, :], qT_sb.rearrange("p a b -> p (a b)"))
                nc.sync.dma_start(dbg["dbg_k"][:, :], kpad_sb.rearrange("p a b -> p (a b)"))
                nc.sync.dma_start(dbg["dbg_v"][:, :], v_sb.rearrange("p a b -> p (a b)"))
                nc.sync.dma_start(dbg["dbg_ao"][:, :], aoT_sb.rearrange("p a b -> p (a b)"))


def _ensure_ntff_hook():
    """Install the NTFF profiling hook that bass_utils expects under axon."""
    import sys
    import types

    try:
        from antenv.axon_hooks import get_axon_ntff_profile_hook  # noqa: F401

        return
    except ImportError:
        pass
    import antenv

    mod = types.ModuleType("antenv.axon_hooks")
    _hook = [None]
    mod.set_axon_ntff_profile_hook = lambda h: _hook.__setitem__(0, h)
    mod.get_axon_ntff_profile_hook = lambda: _hook[0]
    sys.modules["antenv.axon_hooks"] = mod
    antenv.axon_hooks = mod
    try:
        from trn_agent_boot.trn_boot import _ntff_profile_via_ctypes

        mod.set_axon_ntff_profile_hook(
            _ntff_profile_via_ctypes("/opt/axon/libaxon_pjrt.so")
        )
    except Exception:
        pass


_NC_CACHE = None


def _get_nc():
    global _NC_CACHE
    if _NC_CACHE is None:
        _NC_CACHE = _build_kernel()
    return _NC_CACHE


def kernel(x, w_qkv, w_out, b_out, suppress, _trace=False):
    global LAST_RESULTS
    x = np.asarray(x, dtype=np.float32)
    w_qkv = np.asarray(w_qkv, dtype=np.float32)
    w_out_np = np.asarray(w_out, dtype=np.float32)
    b_out_np = np.asarray(b_out, dtype=np.float32)
    c = float(np.asarray(suppress)) / (1.0 + 1e-6)

    bf = ml_dtypes.bfloat16
    w_qk_f = np.ascontiguousarray(w_qkv[:, : 2 * DIM]).copy()
    w_qk_f[:, :DIM] *= SCALE  # fold softmax scale into Q so Exp runs scale-free
    w_qk_b = w_qk_f.astype(bf)
    w_v_b = np.ascontiguousarray(w_qkv[:, 2 * DIM:] * c).astype(bf)
    w_out_b = w_out_np.astype(bf)
    b_out_b = b_out_np.reshape(1, DIM).astype(bf)

    nc = _get_nc()
    in_maps = []
    for core in range(N_CORES):
        xs = x[core * B_PC:(core + 1) * B_PC].reshape(T, DIM)
        xT_b = np.ascontiguousarray(xs.T).astype(bf)
        in_maps.append(
            {
                "xT": xT_b,
                "w_qk": w_qk_b,
                "w_v": w_v_b,
                "w_out": w_out_b,
                "b_out": b_out_b,
            }
        )

    if _trace:
        _ensure_ntff_hook()
    res = run_bass_kernel_spmd(
        nc, in_maps, core_ids=list(range(N_CORES)), trace=_trace
    )
    LAST_RESULTS = res
    outs = [res.results[cc]["out"].reshape(B_PC, N, DIM) for cc in range(N_CORES)]
    return np.concatenate(outs, axis=0)


# revision 30
# speedup vs baseline: 1.0008x; 1.0008x over previous
"""Trainium2 Bass kernel for nn_Attention_5609227288590 (sparse_attention).

Math: the reference's suppress branch with THRES=1.0 has an all-True mask
(every attn value <= its row max), so it reduces exactly to

    attn' = suppress * attn^2 / (rowsum(attn) + 1e-6)

with rowsum(attn) == 1 up to fp rounding of the softmax itself.  Writing
P = exp(S) (no max subtraction needed: |S| <= ~4 for this distribution),
Z_i = sum_j P_ij:

    out_head[i, :] = c * (P∘P) @ V / Z_i^2 ,   c = suppress / (1 + 1e-6)

Per-core layout (data-parallel over batch, 2 batches/core):
  - qT (channels x tokens) computed with w_qkv as stationary, x^T as moving.
    The softmax scale is folded into the Q weights host-side.
  - K^T is stored ZERO-PADDED to K=128: per head pair, two [128, T] tiles
    (rows 0:64 = even head's K^T over zeros / rows 64:128 = odd head's over
    zeros).  S^T matmuls then run with a full-K=128 stationary: measured
    216ns/512-col vs 427+ for the K=64 form (the PE streams K<=64
    stationaries at half rate).
  - V computed in (tokens x channels) layout -> V slices are direct lhsT for
    the PV matmul; P2^T is the moving operand (contraction j on partitions).
  - Z by ones[128,64]-stationary matmuls col-packed 2 heads/bank, giving Z
    broadcast across 64 partitions, matching the PV psum layout.  (gpsimd
    partition_all_reduce measured 13us/[128,2048] on HW - unusable.)
  - Exp(S) is the ONLY ScalarE work; P^2 is one fused [128,2048] DVE square
    per jt, and 1/Z^2 is DVE reciprocal_approx_fast + two multiplies.
  - Slots run BATCH-OUTER so each batch's attn-out completes mid-kernel;
    its out-projection token-tiles are injected one-per-slot into the
    pipeline (psum from the "s" tag ring, keeping ring parity) so only the
    last batch's 8 tiles remain as tail.  Each pipeline step emits
    zpv(k-LAG) BEFORE s_exp_sq(k) (measured ~2us faster than S-first).
  - ~12 warmup matmuls on a zeroed tile run during the input-DMA window so
    the PE p-state is at full clock (2.4GHz) when real work arrives.

Measured (full clock): PE busy ~373us vs a 369us bf16 column-streaming
floor; exec ~414-416us.  Dead ends verified on HW: gpsimd
partition_all_reduce 13us/[128,2048]; any concurrent gpsimd bulk op slows
DVE ~7x (SBUF contention); bf16->fp8 casts for DoubleRow-Z exceed DVE
slack; fp8 anywhere upstream of exp or in P^2/V costs ~3.5% output error
vs the 2e-2 budget.
"""

import numpy as np
import ml_dtypes

import concourse.bass as bass
import concourse.mybir as mybir
import concourse.tile as tile
from concourse import bacc
from concourse.bass_utils import run_bass_kernel_spmd

BF16 = mybir.dt.bfloat16
F32 = mybir.dt.float32
AF = mybir.ActivationFunctionType

N_CORES = 8
B = 16
N = 1024
DIM = 768
HEADS = 12
DH = 64
B_PC = B // N_CORES          # 2 batches per core
T = B_PC * N                 # 2048 tokens per core
PAIRS = HEADS // 2           # 6 head pairs
KT = DIM // 128              # 6 contraction tiles for projections
SCALE = DH ** -0.5           # 0.125
JTS = N // 128               # 8 j-tiles per attention step

LAST_RESULTS = None  # BassKernelResults of the last run (for test.py)


def _patch_act_tables():
    """Pin all activations to the natural_log_exp_and_others table set."""
    import concourse.hw_specs as hw_specs

    if getattr(bacc, "_act_tables_patched", False):
        return
    orig = hw_specs.get_activation_tables

    def patched(module_arch):
        tabs = orig(module_arch)
        return {
            name: (funcs if name == "natural_log_exp_and_others" else set())
            for name, funcs in tabs.items()
        }

    bacc.get_activation_tables = patched
    bacc._act_tables_patched = True


def _build_kernel():
    _patch_act_tables()
    nc = bacc.Bacc("TRN2", target_bir_lowering=False, debug=False)

    xT = nc.dram_tensor("xT", [DIM, T], BF16, kind="ExternalInput")
    w_qk = nc.dram_tensor("w_qk", [DIM, 2 * DIM], BF16, kind="ExternalInput")
    w_v = nc.dram_tensor("w_v", [DIM, DIM], BF16, kind="ExternalInput")
    w_out = nc.dram_tensor("w_out", [DIM, DIM], BF16, kind="ExternalInput")
    b_out = nc.dram_tensor("b_out", [1, DIM], BF16, kind="ExternalInput")
    out = nc.dram_tensor("out", [T, DIM], F32, kind="ExternalOutput")
    import os
    dbg = None
    if os.environ.get("KDEBUG") == "1":
        dbg = {
            "dbg_q": nc.dram_tensor("dbg_q", [128, PAIRS * T], BF16, kind="ExternalOutput"),
            "dbg_k": nc.dram_tensor("dbg_k", [128, 2 * PAIRS * T], BF16, kind="ExternalOutput"),
            "dbg_v": nc.dram_tensor("dbg_v", [128, (T // 128) * DIM], BF16, kind="ExternalOutput"),
            "dbg_ao": nc.dram_tensor("dbg_ao", [128, KT * T], BF16, kind="ExternalOutput"),
        }

    with tile.TileContext(nc) as tc:
        _body(nc, tc, xT, w_qk, w_v, w_out, b_out, out, dbg)
    nc.compile()
    return nc


def _body(nc, tc, xT, w_qk, w_v, w_out, b_out, out, dbg=None):
    from contextlib import ExitStack

    ctx = ExitStack()
    with ctx:
        singles = ctx.enter_context(tc.tile_pool(name="singles", bufs=1))

        # ---- persistent SBUF tensors ----
        w_v_sb = singles.tile([128, KT, DIM], BF16)
        w_out_sb = singles.tile([128, KT, DIM], BF16)
        b_out_sb = singles.tile([1, DIM], BF16)
        bias_bcast = singles.tile([128, DIM], BF16)
        ones64 = singles.tile([128, DH], BF16)
        ones1 = singles.tile([1, 128], BF16)
        qT_sb = singles.tile([128, PAIRS, T], BF16)
        # zero-padded K^T: per pair, index 2h   = [K_h0^T (rows 0:64); 0]
        #                            index 2h+1 = [0; K_h1^T (rows 64:128)]
        kpad_sb = singles.tile([128, 2 * PAIRS, T], BF16)
        v_sb = singles.tile([128, T // 128, DIM], BF16)    # [t, c] layout
        aoT_sb = singles.tile([128, KT, T], BF16)          # attn-outT stacked

        w_qk_r = w_qk.rearrange("(ko p) c -> p ko c", p=128)
        w_v_r = w_v.rearrange("(ko p) c -> p ko c", p=128)
        xt_r = xT.rearrange("(ko p) t -> p ko t", p=128)

        # ---- phase 1+2: projections ----
        with (
            tc.tile_pool(name="proj_in", bufs=1) as proj_in,
            tc.tile_pool(name="proj_ps", bufs=2, space="PSUM") as proj_ps,
        ):
            w_qk_sb = proj_in.tile([128, KT, 2 * DIM], BF16)
            xt_sb = proj_in.tile([128, KT, T], BF16)
            warm = proj_in.tile([128, 640], BF16)
            # zero-fills + warmup first: no input deps, run during DMA wait
            nc.any.memset(warm, 0.0)
            nc.any.memset(ones64, 1.0)
            nc.any.memset(ones1, 1.0)
            for h in range(PAIRS):
                nc.gpsimd.memset(kpad_sb[64:128, 2 * h, :], 0.0)
                nc.gpsimd.memset(kpad_sb[0:64, 2 * h + 1, :], 0.0)
            ps_w = proj_ps.tile([128, T], F32, tag="proj", name="ps_w")
            for r in range(12):
                nc.tensor.matmul(
                    ps_w[:, (r % 4) * 512:(r % 4) * 512 + 512],
                    warm[:, 0:128],
                    warm[:, 128:640],
                    start=True,
                    stop=True,
                )
            # DMA order: first QK weight block, then x chunks (kt-major so
            # the kt-inner projection loop starts as chunks land), then rest.
            nc.sync.dma_start(w_qk_sb[:, :, 0:128], w_qk_r[:, :, 0:128])
            nc.sync.dma_start(xt_sb[:, 0, 0:1024], xt_r[:, 0, 0:1024])
            nc.sync.dma_start(w_qk_sb[:, :, 128:640], w_qk_r[:, :, 128:640])
            nc.sync.dma_start(xt_sb[:, 0, 1024:2048], xt_r[:, 0, 1024:2048])
            for kt in range(1, KT):
                nc.sync.dma_start(xt_sb[:, kt, :], xt_r[:, kt, :])
            nc.sync.dma_start(w_qk_sb[:, :, 640:], w_qk_r[:, :, 640:])
            nc.sync.dma_start(w_v_sb, w_v_r)
            nc.sync.dma_start(
                w_out_sb, w_out.rearrange("(ko p) c -> p ko c", p=128)
            )
            nc.sync.dma_start(b_out_sb, b_out[:, :])

            for mt in range(2 * PAIRS):
                ps = proj_ps.tile([128, T], F32, tag="proj")
                for kt in range(KT):
                    for c in range(T // 512):
                        nc.tensor.matmul(
                            ps[:, c * 512:(c + 1) * 512],
                            w_qk_sb[:, kt, mt * 128:(mt + 1) * 128],
                            xt_sb[:, kt, c * 512:(c + 1) * 512],
                            start=(kt == 0),
                            stop=(kt == KT - 1),
                        )
                if mt < PAIRS:
                    nc.vector.tensor_copy(out=qT_sb[:, mt, :], in_=ps)
                else:
                    h = mt - PAIRS
                    nc.vector.tensor_copy(
                        out=kpad_sb[0:64, 2 * h, :], in_=ps[0:64, :]
                    )
                    nc.vector.tensor_copy(
                        out=kpad_sb[64:128, 2 * h + 1, :], in_=ps[64:128, :]
                    )

            for mt in range(T // 128):
                ps = proj_ps.tile([128, DIM], F32, tag="proj")
                for kt in range(KT):
                    for c0, c1 in ((0, 512), (512, 768)):
                        nc.tensor.matmul(
                            ps[:, c0:c1],
                            xt_sb[:, kt, mt * 128:(mt + 1) * 128],
                            w_v_sb[:, kt, c0:c1],
                            start=(kt == 0),
                            stop=(kt == KT - 1),
                        )
                nc.vector.tensor_copy(out=v_sb[:, mt, :], in_=ps)

            # broadcast the output bias across partitions once (K=1 matmul)
            ps_b = proj_ps.tile([128, DIM], F32, tag="proj")
            for c0, c1 in ((0, 512), (512, 768)):
                nc.tensor.matmul(
                    ps_b[:, c0:c1],
                    ones1[0:1, 0:128],
                    b_out_sb[0:1, c0:c1],
                    start=True,
                    stop=True,
                )
            nc.vector.tensor_copy(out=bias_bcast, in_=ps_b)

        # ---- phase 3: attention, per (batch, head pair) + fused out-proj ----
        with (
            tc.tile_pool(name="pt_pool", bufs=8) as pt_pool,
            tc.tile_pool(name="p2t_pool", bufs=5) as p2t_pool,
            tc.tile_pool(name="z_sb_pool", bufs=1) as z_sb_pool,
            tc.tile_pool(name="f_sb", bufs=2) as f_sb,
            tc.tile_pool(name="s_ps", bufs=2, space="PSUM") as s_ps,
            tc.tile_pool(name="o_ps", bufs=1, space="PSUM") as o_ps,
            tc.tile_pool(name="z_ps", bufs=1, space="PSUM") as z_ps,
        ):
            pts, p2ts = {}, {}
            obank = {}  # (h, b) -> (psum_o, psum_z)

            def s_exp_sq(h, b, jt):
                t0 = b * N
                qT = qT_sb[:, h, t0:t0 + N]
                pt = pt_pool.tile([128, 2 * N], BF16, tag="pt")
                p2t = p2t_pool.tile([128, 2 * N], BF16, tag="p2t")
                pts[(h, b, jt)], p2ts[(h, b, jt)] = pt, p2t
                with tc.high_priority(offset=90):
                    for hh in (1, 0):
                        kp = kpad_sb[
                            :, 2 * h + hh, t0 + jt * 128:t0 + (jt + 1) * 128
                        ]
                        ps = s_ps.tile([128, N], F32, tag="s", name="psS")
                        for c in range(2):
                            nc.tensor.matmul(
                                ps[:, c * 512:(c + 1) * 512],
                                kp,
                                qT[:, c * 512:(c + 1) * 512],
                                start=True,
                                stop=True,
                            )
                        nc.scalar.activation(
                            pt[:, hh * N:(hh + 1) * N], ps, AF.Exp,
                        )
                    # one fused square for both head halves (DVE 2x bf16)
                    nc.vector.tensor_mul(out=p2t, in0=pt, in1=pt)

            def zpv(h, b, jt):
                # skip_group_check: the sim's global zero-region check
                # mishandles base_partition!=0; col-split groups are
                # HW-safe (verified by direct probe).
                if jt == 0:
                    obank[(h, b)] = (
                        o_ps.tile([128, 1024], F32, tag="o", name="psum_o"),
                        z_ps.tile([128, 1024], F32, tag="z", name="psum_z"),
                    )
                psum_o, psum_z = obank[(h, b)]
                pt, p2t = pts.pop((h, b, jt)), p2ts.pop((h, b, jt))
                vt = v_sb[:, b * 8 + jt, :]
                first, last = jt == 0, jt == JTS - 1
                for hh in (1, 0):
                    d0, d1 = hh * 64, hh * 64 + 64
                    ch0 = h * 128 + hh * 64
                    for c in range(2):
                        nc.tensor.matmul(
                            psum_z[d0:d1, c * 512:(c + 1) * 512],
                            ones64,
                            pt[:, hh * N + c * 512:hh * N + (c + 1) * 512],
                            start=first,
                            stop=last,
                            skip_group_check=True,
                        )
                    for c in range(2):
                        nc.tensor.matmul(
                            psum_o[d0:d1, c * 512:(c + 1) * 512],
                            vt[:, ch0:ch0 + 64],
                            p2t[:, hh * N + c * 512:hh * N + (c + 1) * 512],
                            start=first,
                            stop=last,
                            skip_group_check=True,
                        )
                if last:
                    psum_o, psum_z = obank.pop((h, b))
                    zinv = z_sb_pool.tile([128, 1024], F32, tag="zinv")
                    otmp = z_sb_pool.tile([128, 1024], BF16, tag="otmp")
                    with tc.high_priority(offset=90):
                        nc.vector.reciprocal_approx_fast(zinv, psum_z)
                        nc.vector.tensor_mul(out=otmp, in0=psum_o, in1=zinv)
                        nc.vector.tensor_mul(
                            out=aoT_sb[:, h, b * N:(b + 1) * N],
                            in0=otmp,
                            in1=zinv,
                        )

            def out_proj(mt):
                # one token-tile of the final projection; psum comes from the
                # "s" tag ring so psum stays within 8 banks (parity: exactly
                # one of these per pipeline step).
                ps = s_ps.tile([128, N], F32, tag="s", name="psF")
                for c0, c1 in ((0, 512), (512, 768)):
                    for kt in range(KT):
                        nc.tensor.matmul(
                            ps[:, c0:c1],
                            aoT_sb[:, kt, mt * 128:(mt + 1) * 128],
                            w_out_sb[:, kt, c0:c1],
                            start=(kt == 0),
                            stop=(kt == KT - 1),
                        )
                o_sb = f_sb.tile([128, DIM], F32, tag="fo")
                nc.vector.tensor_add(out=o_sb, in0=ps[:, 0:DIM], in1=bias_bcast)
                nc.sync.dma_start(out[mt * 128:(mt + 1) * 128, :], o_sb)

            # global slot pipeline, batch-outer: zpv lags s_exp_sq by LAG
            # slots; once a batch's last zpv ran, its out-proj token tiles
            # are fed one-per-step into the same pipeline.
            LAG = 2
            slots = [
                (h, b, jt)
                for b in range(B_PC)
                for h in range(PAIRS)
                for jt in range(JTS)
            ]
            pending_f = []  # out-proj token tiles ready to issue
            for k in range(len(slots) + LAG):
                if k >= LAG:
                    h, b, jt = slots[k - LAG]
                    zpv(h, b, jt)
                    if h == PAIRS - 1 and jt == JTS - 1:
                        pending_f.extend(range(b * 8, (b + 1) * 8))
                if k < len(slots):
                    s_exp_sq(*slots[k])
                if pending_f:
                    out_proj(pending_f.pop(0))
            while pending_f:
                out_proj(pending_f.pop(0))
            if dbg is not None:
                nc.sync.dma_start(dbg["dbg_q"][:, :], qT_sb.rearrange("p a b -> p (a b)"))
                nc.sync.dma_start(dbg["dbg_k"][:, :], kpad_sb.rearrange("p a b -> p (a b)"))
                nc.sync.dma_start(dbg["dbg_v"][:, :], v_sb.rearrange("p a b -> p (a b)"))
                nc.sync.dma_start(dbg["dbg_ao"][:, :], aoT_sb.rearrange("p a b -> p (a b)"))


def _ensure_ntff_hook():
    """Install the NTFF profiling hook that bass_utils expects under axon."""
    import sys
    import types

    try:
        from antenv.axon_hooks import get_axon_ntff_profile_hook  # noqa: F401

        return
    except ImportError:
        pass
    import antenv

    mod = types.ModuleType("antenv.axon_hooks")
    _hook = [None]
    mod.set_axon_ntff_profile_hook = lambda h: _hook.__setitem__(0, h)
    mod.get_axon_ntff_profile_hook = lambda: _hook[0]
    sys.modules["antenv.axon_hooks"] = mod
    antenv.axon_hooks = mod
    try:
        from trn_agent_boot.trn_boot import _ntff_profile_via_ctypes

        mod.set_axon_ntff_profile_hook(
            _ntff_profile_via_ctypes("/opt/axon/libaxon_pjrt.so")
        )
    except Exception:
        pass


_NC_CACHE = None


def _get_nc():
    global _NC_CACHE
    if _NC_CACHE is None:
        _NC_CACHE = _build_kernel()
    return _NC_CACHE


def kernel(x, w_qkv, w_out, b_out, suppress, _trace=False):
    global LAST_RESULTS
    x = np.asarray(x, dtype=np.float32)
    w_qkv = np.asarray(w_qkv, dtype=np.float32)
    w_out_np = np.asarray(w_out, dtype=np.float32)
    b_out_np = np.asarray(b_out, dtype=np.float32)
    c = float(np.asarray(suppress)) / (1.0 + 1e-6)

    bf = ml_dtypes.bfloat16
    w_qk_f = np.ascontiguousarray(w_qkv[:, : 2 * DIM]).copy()
    w_qk_f[:, :DIM] *= SCALE  # fold softmax scale into Q so Exp runs scale-free
    w_qk_b = w_qk_f.astype(bf)
    w_v_b = np.ascontiguousarray(w_qkv[:, 2 * DIM:] * c).astype(bf)
    w_out_b = w_out_np.astype(bf)
    b_out_b = b_out_np.reshape(1, DIM).astype(bf)

    nc = _get_nc()
    in_maps = []
    for core in range(N_CORES):
        xs = x[core * B_PC:(core + 1) * B_PC].reshape(T, DIM)
        xT_b = np.ascontiguousarray(xs.T).astype(bf)
        in_maps.append(
            {
                "xT": xT_b,
                "w_qk": w_qk_b,
                "w_v": w_v_b,
                "w_out": w_out_b,
                "b_out": b_out_b,
            }
        )

    if _trace:
        _ensure_ntff_hook()
    res = run_bass_kernel_spmd(
        nc, in_maps, core_ids=list(range(N_CORES)), trace=_trace
    )
    LAST_RESULTS = res
    outs = [res.results[cc]["out"].reshape(B_PC, N, DIM) for cc in range(N_CORES)]
    return np.concatenate(outs, axis=0)
